# revision 9
# baseline (speedup 1.0000x reference)
"""CIN (Compressed Interaction Network) forward kernel for Trainium2.

Data-parallel over 8 NeuronCores: batch dim B=2048 sharded 256/core, conv
weights replicated. No cross-device communication.

Per-core layout: (channels, n) with n = (b_local, d) flattened to 8192
columns, processed in chunks of NC=256 columns.

Strategy (hybrid fp8/fp16, engine-balanced):
- Layer 0 (x (x) x, symmetric-folded to 780 pairs padded to 896 rows) is
  host-precomputed as an fp16 interaction tensor and fed straight to 7
  fp16 K=128 matmul tiles per output half. No device elementwise work.
- Layers 1/2 contract K = 128*39. Fields f < NP8 run as fp8e4 DoubleRow
  matmuls (K=256/instr, 2x PE throughput); fields f >= NP8 stay fp16.
  Weights are scaled by 64 into e4m3's normal range; hidden activations
  un-scale via ScalarE's scale=1/64, direct outputs keep the 64x (bias
  also 64x) and the host divides at the end.
- The fp8 rhs (hidden*x products) is built two ways to balance engines:
  DVE fp16 products at 2x then ScalarE converts to fp8 (f < NTS), or DVE
  multiplies straight to fp8 at 1x (NTS <= f < NP8). The fp16-matmul
  products (f >= NP8) run on GPSIMD. x-broadcast tiles come from
  stride-0 DMA on two rings.
- Pipeline rotation L0(k+2) | L1(k+1) | L2(k): every hidden->products->
  matmul chain gets a full iteration (~11us) of independent PE work.
"""

import sys

if "/opt/trn_rl_repo" not in sys.path:
    sys.path.insert(0, "/opt/trn_rl_repo")

from contextlib import ExitStack

import numpy as np
import ml_dtypes

import concourse.bacc as bacc
import concourse.bass as bass
import concourse.mybir as mybir
import concourse.tile as tile
from concourse import bass_utils

# Problem shapes (hardcoded per contest rules)
B, F, D = 2048, 39, 32
O = 256          # conv output channels per layer
H = 128          # hidden channels fed to layers 1,2
NCORES = 8
B_LOC = B // NCORES          # 256 batches per core
N_LOC = B_LOC * D            # 8192 columns per core

NC = 256                     # columns per chunk
NB = NC // D                 # batches per chunk (8)
NPAIR = F * (F + 1) // 2     # 780 folded layer-0 pairs
QG = (NPAIR + 127) // 128    # 7 K-tiles
Q = QG * 128                 # 896 padded rows

# hybrid split for layers 1/2
NP8 = 30                     # f < NP8 -> fp8 DoubleRow (must be even)
NTS = 20                     # f < NTS -> two-step (DVE fp16 prod + ACT cvt)
NDIR = NP8 - NTS             # direct DVE fp16*fp16->fp8 products
NF16 = F - NP8               # fp16 matmul fields (GPSIMD products)
NPAIRS8 = NP8 // 2           # DoubleRow pair count
WS = 64.0                    # weight scale (e4m3 subnormal dodge)

F16 = mybir.dt.float16
F32 = mybir.dt.float32
F8 = mybir.dt.float8e4
AF = mybir.ActivationFunctionType
DR = mybir.MatmulPerfMode.DoubleRow

TRACE = False
_LAST_RESULTS = None


def build_module(b_loc=B_LOC, nc_cols=NC):
    n_loc = b_loc * D
    nchunk = n_loc // nc_cols
    nb = nc_cols // D
    assert n_loc % nc_cols == 0 and nc_cols % D == 0

    nc = bacc.Bacc("TRN2", target_bir_lowering=False, debug=False)

    xtc = nc.dram_tensor("xtc", (nchunk, F, nc_cols), F16, kind="ExternalInput").ap()
    rhs0 = nc.dram_tensor("rhs0", (nchunk, 128, QG * nc_cols), F16, kind="ExternalInput").ap()
    wt0 = nc.dram_tensor("wt0", (128, QG * O), F16, kind="ExternalInput").ap()
    wt8 = nc.dram_tensor("wt8", (2, 128, NP8 * O), F8, kind="ExternalInput").ap()
    wt16 = nc.dram_tensor("wt16", (2, 128, NF16 * O), F16, kind="ExternalInput").ap()
    biases = nc.dram_tensor("biases", (128, 8), F32, kind="ExternalInput").ap()
    out = nc.dram_tensor("out", (128, 4, b_loc), F32, kind="ExternalOutput").ap()

    with tile.TileContext(nc) as tc, ExitStack() as ctx:
        const = ctx.enter_context(tc.tile_pool(name="const", bufs=1))
        t_pool = ctx.enter_context(tc.tile_pool(name="tpool", bufs=3))
        r0_pool = ctx.enter_context(tc.tile_pool(name="r0pool", bufs=2))
        rhs_pool = ctx.enter_context(tc.tile_pool(name="rhspool", bufs=2))
        tmp_pool = ctx.enter_context(tc.tile_pool(name="tmppool", bufs=2))
        hid_pool = ctx.enter_context(tc.tile_pool(name="hidpool", bufs=2))
        dt_pool = ctx.enter_context(tc.tile_pool(name="dtpool", bufs=3))
        psum_pool = ctx.enter_context(tc.tile_pool(name="psum", bufs=8, space="PSUM"))

        # --- resident tensors ---
        wt0_sb = const.tile([128, QG, O], F16)
        wt8_sb = [const.tile([128, NPAIRS8, 2, O], F8, name=f"w8_{l}") for l in range(2)]
        wt16_sb = [const.tile([128, NF16, O], F16, name=f"w16_{l}") for l in range(2)]
        bias_sb = const.tile([128, 8], F32)
        out_sb = const.tile([128, 4, b_loc], F32)

        nc.sync.dma_start(bias_sb[:], biases)
        nc.sync.dma_start(wt0_sb[:], wt0.rearrange("p (g o) -> p g o", o=O))
        # PE warmup: dep-free matmuls keep the p-state high through the
        # input-load window.
        warm_ps = psum_pool.tile([128, nc_cols], F32, tag="ps", name="warm_ps")
        for _ in range(72):
            nc.tensor.matmul(
                warm_ps[0:8, 0:8], bias_sb[:, 0:8], bias_sb[:, 0:8],
                start=True, stop=True,
            )
        for l in range(2):
            nc.scalar.dma_start(
                wt8_sb[l][:],
                wt8[l].rearrange("p (g j o) -> p g j o", g=NPAIRS8, j=2),
            )
            nc.scalar.dma_start(
                wt16_sb[l][:], wt16[l].rearrange("p (f o) -> p f o", o=O)
            )

        def load_T(j):
            """x0t rows broadcast to 128 partitions, split across two rings."""
            t_t = t_pool.tile([128, F, nc_cols], F16, tag="T", name=f"t_{j}")
            h1 = F // 2
            nc.sync.dma_start(
                t_t[:, 0:h1, :], xtc[j, 0:h1, :].partition_broadcast(128)
            )
            nc.scalar.dma_start(
                t_t[:, h1:F, :], xtc[j, h1:F, :].partition_broadcast(128)
            )
            return t_t

        def load_rhs0(j):
            r_t = r0_pool.tile([128, QG, nc_cols], F16, tag="r0", name=f"r0_{j}")
            nc.sync.dma_start(
                r_t[:], rhs0[j].rearrange("p (g i) -> p g i", i=nc_cols)
            )
            return r_t

        def mm_l0(j, r0_t, m, ps):
            for g in range(QG):
                nc.tensor.matmul(
                    ps[:],
                    wt0_sb[:, g, m * 128 : (m + 1) * 128],
                    r0_t[:, g, :],
                    start=(g == 0),
                    stop=(g == QG - 1),
                )

        def mm_l12(l, m, ps, rhs8_t, rhs16_t):
            """Hybrid matmul stream for layer l+1, output half m."""
            for g in range(NPAIRS8):
                nc.tensor.matmul(
                    ps[:],
                    wt8_sb[l][:, g, :, m * 128 : (m + 1) * 128],  # [128, 2, 128]
                    rhs8_t[:, g],                                 # [128, 2, nc]
                    start=(g == 0),
                    stop=False,
                    perf_mode=DR,
                )
            for t in range(NF16):
                nc.tensor.matmul(
                    ps[:],
                    wt16_sb[l][:, t, m * 128 : (m + 1) * 128],
                    rhs16_t[:, t, :],
                    start=False,
                    stop=(t == NF16 - 1),
                )

        def products(j, l, hid, t_t):
            """Build rhs8 [128, NPAIRS8, 2, nc] fp8 + rhs16 [128, NF16, nc]."""
            rhs8_t = rhs_pool.tile(
                [128, NPAIRS8, 2, nc_cols], F8, tag=f"rhs8_{l}", name=f"r8_{j}_{l}"
            )
            rhs8_flat = rhs8_t[:].rearrange("p g j i -> p (g j) i")
            rhs16_t = rhs_pool.tile(
                [128, NF16, nc_cols], F16, tag=f"rhs16_{l}", name=f"r16_{j}_{l}"
            )
            # GPSIMD: fp16 products for the fp16 matmul fields
            nc.gpsimd.tensor_mul(
                rhs16_t[:],
                hid[:].unsqueeze(1).broadcast_to((128, NF16, nc_cols)),
                t_t[:, NP8:F, :],
            )
            # two-step: DVE fp16 products (2x) in two groups -> ACT converts
            g1 = NTS // 2
            tmp_t = tmp_pool.tile(
                [128, NTS, nc_cols], F16, tag="tmp", bufs=3, name=f"tmp_{j}_{l}"
            )
            for a, bnd in ((0, g1), (g1, NTS)):
                nc.vector.tensor_mul(
                    tmp_t[:, a:bnd, :],
                    hid[:].unsqueeze(1).broadcast_to((128, bnd - a, nc_cols)),
                    t_t[:, a:bnd, :],
                )
                nc.scalar.activation(
                    rhs8_flat[:, a:bnd, :], tmp_t[:, a:bnd, :], AF.Copy
                )
            # direct: DVE fp16*fp16 -> fp8 at 1x
            nc.vector.tensor_mul(
                rhs8_flat[:, NTS:NP8, :],
                hid[:].unsqueeze(1).broadcast_to((128, NDIR, nc_cols)),
                t_t[:, NTS:NP8, :],
            )
            return rhs8_t, rhs16_t

        def hidden_act(j, l, ps):
            h_t = hid_pool.tile([128, nc_cols], F16, tag="hid", name=f"h_{j}_{l}")
            nc.scalar.activation(
                h_t[:], ps[:], AF.Relu,
                bias=bias_sb[:, 2 * l + 1 : 2 * l + 2], scale=1.0 / WS,
            )
            return h_t

        def direct_relu(j, slot, ps, bias_col, dt_t):
            # dt = relu(ps + 64*b) = 64 * relu(ps/64 + b); host divides by 64
            nc.scalar.activation(
                dt_t[:, slot, :], ps[:], AF.Relu,
                bias=bias_sb[:, bias_col : bias_col + 1],
            )

        # ---- prologue ----
        t_tiles = {0: load_T(0)}
        r0_tiles = {0: load_rhs0(0)}
        dt_tiles = {}
        state = {}   # per-chunk: rhs8/rhs16 tiles for next layer

        for i in range(nchunk + 2):
            k0, k1, k2 = i, i - 1, i - 2
            # prefetch next chunk inputs
            if k0 + 1 < nchunk:
                t_tiles[k0 + 1] = load_T(k0 + 1)
                r0_tiles[k0 + 1] = load_rhs0(k0 + 1)

            if k0 < nchunk:
                dt_t = dt_pool.tile(
                    [128, 4, nc_cols], F16, tag="dt", name=f"dt_{k0}"
                )
                dt_tiles[k0] = dt_t
                r0_t = r0_tiles.pop(k0)
                ps_b = psum_pool.tile([128, nc_cols], F32, tag="ps", name=f"ps_{k0}_0b")
                mm_l0(k0, r0_t, 1, ps_b)
                h0 = hidden_act(k0, 0, ps_b)
                ps_a = psum_pool.tile([128, nc_cols], F32, tag="ps", name=f"ps_{k0}_0a")
                mm_l0(k0, r0_t, 0, ps_a)
                state[(k0, 1)] = products(k0, 0, h0, t_tiles[k0])
                direct_relu(k0, 0, ps_a, 0, dt_t)

            if 0 <= k1 < nchunk:
                rhs8_t, rhs16_t = state.pop((k1, 1))
                ps1 = psum_pool.tile([128, nc_cols], F32, tag="ps", name=f"ps_{k1}_1b")
                mm_l12(0, 1, ps1, rhs8_t, rhs16_t)
                h1 = hidden_act(k1, 1, ps1)
                ps0 = psum_pool.tile([128, nc_cols], F32, tag="ps", name=f"ps_{k1}_1a")
                mm_l12(0, 0, ps0, rhs8_t, rhs16_t)
                state[(k1, 2)] = products(k1, 1, h1, t_tiles[k1])
                direct_relu(k1, 1, ps0, 2, dt_tiles[k1])

            if 0 <= k2 < nchunk:
                rhs8_t, rhs16_t = state.pop((k2, 2))
                ps20 = psum_pool.tile([128, nc_cols], F32, tag="ps", name=f"ps_{k2}_2a")
                mm_l12(1, 0, ps20, rhs8_t, rhs16_t)
                ps21 = psum_pool.tile([128, nc_cols], F32, tag="ps", name=f"ps_{k2}_2b")
                mm_l12(1, 1, ps21, rhs8_t, rhs16_t)
                dt_t = dt_tiles.pop(k2)
                direct_relu(k2, 2, ps20, 4, dt_t)
                direct_relu(k2, 3, ps21, 5, dt_t)
                nc.vector.tensor_reduce(
                    out_sb[:, :, k2 * nb : (k2 + 1) * nb],
                    dt_t[:].rearrange("p s (b d) -> p s b d", d=D),
                    axis=mybir.AxisListType.X,
                    op=mybir.AluOpType.add,
                )
                t_tiles.pop(k2, None)

        nc.sync.dma_start(out, out_sb[:])

    nc.compile()
    return nc


def _e4m3(a):
    a = np.clip(np.asarray(a, np.float32), -240.0, 240.0)
    return a.astype(ml_dtypes.float8_e4m3fn)


def _pack_inputs(field_embeddings, w0, b0, w1, b1, w2, b2, b_loc=B_LOC, nc_cols=NC):
    x = np.asarray(field_embeddings, dtype=np.float32)
    w0 = np.asarray(w0, dtype=np.float32)
    ncores = x.shape[0] // b_loc
    n_loc = b_loc * D
    nchunk = n_loc // nc_cols

    # layer-0 folded pairs
    hq = np.array([h for f_ in range(F) for h in range(f_ + 1)])
    fq = np.array([f_ for f_ in range(F) for h in range(f_ + 1)])
    w0r = w0.reshape(O, F, F)
    wf = w0r[:, hq, fq] + np.where(hq == fq, 0.0, w0r[:, fq, hq])   # (O, NPAIR)
    wf_pad = np.zeros((O, Q), dtype=np.float32)
    wf_pad[:, :NPAIR] = wf * WS
    wt0h = np.ascontiguousarray(
        wf_pad.reshape(O, QG, 128).transpose(2, 1, 0).reshape(128, QG * O)
    ).astype(np.float16)

    # layers 1/2: fp8 pair-packed weights [l][p=h, g, j, m, t] and fp16 tail
    def pack_l(w):
        a = np.asarray(w, np.float32).reshape(O, H, F).transpose(1, 2, 0) * WS  # (h,f,o)
        w8 = _e4m3(a[:, :NP8, :])                       # (h, NP8, O) = (p, g, j, o)
        w8 = np.ascontiguousarray(w8.reshape(H, NP8 * O))
        w16_ = np.ascontiguousarray(a[:, NP8:, :].reshape(H, NF16 * O)).astype(np.float16)
        return w8, w16_

    w8_1, w16_1 = pack_l(w1)
    w8_2, w16_2 = pack_l(w2)
    wt8h = np.stack([w8_1, w8_2])
    wt16h = np.stack([w16_1, w16_2])

    biash = np.zeros((128, 8), dtype=np.float32)
    for li, bvec in enumerate([b0, b1, b2]):
        bvec = np.asarray(bvec, dtype=np.float32)
        biash[:, 2 * li] = bvec[0:128] * WS          # direct half (kept scaled)
        biash[:, 2 * li + 1] = bvec[128:256]         # hidden half (scale=1/64)
    biash[:, 5] = np.asarray(b2, np.float32)[128:256] * WS  # L2 m1 is direct too

    in_maps = []
    for c in range(ncores):
        xc = x[c * b_loc : (c + 1) * b_loc]
        x0t = xc.transpose(1, 0, 2).reshape(F, n_loc)
        xtc = x0t.reshape(F, nchunk, nc_cols).transpose(1, 0, 2)
        x0t16 = x0t.astype(np.float16)
        # host-side layer-0 interactions (fp16 products of fp16 x)
        r0 = np.zeros((Q, n_loc), dtype=np.float16)
        r0[:NPAIR] = (x0t16[hq].astype(np.float32) * x0t16[fq].astype(np.float32)).astype(np.float16)
        r0p = r0.reshape(QG, 128, nchunk, nc_cols).transpose(2, 1, 0, 3)
        r0p = np.ascontiguousarray(r0p.reshape(nchunk, 128, QG * nc_cols))
        in_maps.append(
            {
                "xtc": np.ascontiguousarray(xtc).astype(np.float16),
                "rhs0": r0p,
                "wt0": wt0h,
                "wt8": wt8h,
                "wt16": wt16h,
                "biases": biash,
            }
        )
    return in_maps


_MODULE = None


def kernel(field_embeddings, w0, b0, w1, b1, w2, b2):
    global _MODULE, _LAST_RESULTS
    if _MODULE is None:
        _MODULE = build_module()
    nc = _MODULE
    in_maps = _pack_inputs(field_embeddings, w0, b0, w1, b1, w2, b2)
    res = bass_utils.run_bass_kernel_spmd(
        nc, in_maps, core_ids=list(range(NCORES)), trace=TRACE
    )
    _LAST_RESULTS = res
    outs = []
    for c in range(NCORES):
        o = res.results[c]["out"]                  # (128, 4, B_LOC) fp32
        full = o.transpose(1, 0, 2).reshape(512, B_LOC) * (1.0 / WS)
        outs.append(full.T)                        # (B_LOC, 512)
    return np.ascontiguousarray(np.concatenate(outs, axis=0), dtype=np.float32)


# revision 11
# speedup vs baseline: 1.0798x; 1.0798x over previous
"""CIN (Compressed Interaction Network) forward kernel for Trainium2.

Data-parallel over 8 NeuronCores: batch dim B=2048 sharded 256/core, conv
weights replicated. No cross-device communication.

Per-core layout: (channels, n) with n = (b_local, d) flattened to 8192
columns, processed in chunks of NC=256 columns.

Strategy (hybrid fp8/fp16, engine-balanced):
- Layer 0 (x (x) x, symmetric-folded to 780 pairs padded to 896 rows) is
  host-precomputed as an fp16 interaction tensor and fed straight to 7
  fp16 K=128 matmul tiles per output half. No device elementwise work.
- Layers 1/2 contract K = 128*39. Fields f < NP8 run as fp8e4 DoubleRow
  matmuls (K=256/instr, 2x PE throughput); fields f >= NP8 stay fp16.
  Weights are scaled by 64 into e4m3's normal range; hidden activations
  un-scale via ScalarE's scale=1/64, direct outputs keep the 64x (bias
  also 64x) and the host divides at the end.
- The fp8 rhs (hidden*x products) is built two ways to balance engines:
  DVE fp16 products at 2x then ScalarE converts to fp8 (f < NTS), or DVE
  multiplies straight to fp8 at 1x (NTS <= f < NP8). The fp16-matmul
  products (f >= NP8) run on GPSIMD. x-broadcast tiles come from
  stride-0 DMA on two rings.
- Pipeline rotation L0(k+2) | L1(k+1) | L2(k): every hidden->products->
  matmul chain gets a full iteration (~11us) of independent PE work.
"""

import sys

if "/opt/trn_rl_repo" not in sys.path:
    sys.path.insert(0, "/opt/trn_rl_repo")

from contextlib import ExitStack

import numpy as np
import ml_dtypes

import concourse.bacc as bacc
import concourse.bass as bass
import concourse.mybir as mybir
import concourse.tile as tile
from concourse import bass_utils

# Problem shapes (hardcoded per contest rules)
B, F, D = 2048, 39, 32
O = 256          # conv output channels per layer
H = 128          # hidden channels fed to layers 1,2
NCORES = 8
B_LOC = B // NCORES          # 256 batches per core
N_LOC = B_LOC * D            # 8192 columns per core

NC = 256                     # columns per chunk
NB = NC // D                 # batches per chunk (8)
NPAIR = F * (F + 1) // 2     # 780 folded layer-0 pairs
QG = (NPAIR + 127) // 128    # 7 K-tiles
Q = QG * 128                 # 896 padded rows

# hybrid split for layers 1/2
NP8 = 26                     # f < NP8 -> fp8 DoubleRow (must be even)
NTS = NP8                    # all fp8 rhs via two-step (DVE fp16 prod + ACT cvt);
                             # direct DVE fp16->fp8 measured 2.4ns/elem - useless
NGPS = 5                     # fp16-matmul fields built on GPSIMD (2.5ns/elem)
NF16 = F - NP8               # fp16 matmul fields (DVE builds NF16-NGPS of them)
NPAIRS8 = NP8 // 2           # DoubleRow pair count
WS = 64.0                    # weight scale (e4m3 subnormal dodge)

F16 = mybir.dt.float16
F32 = mybir.dt.float32
F8 = mybir.dt.float8e4
AF = mybir.ActivationFunctionType
DR = mybir.MatmulPerfMode.DoubleRow

TRACE = False
_LAST_RESULTS = None


def build_module(b_loc=B_LOC, nc_cols=NC):
    n_loc = b_loc * D
    nchunk = n_loc // nc_cols
    nb = nc_cols // D
    assert n_loc % nc_cols == 0 and nc_cols % D == 0

    nc = bacc.Bacc("TRN2", target_bir_lowering=False, debug=False)

    xtc = nc.dram_tensor("xtc", (nchunk, F, nc_cols), F16, kind="ExternalInput").ap()
    rhs0 = nc.dram_tensor("rhs0", (nchunk, 128, QG * nc_cols), F16, kind="ExternalInput").ap()
    wt0 = nc.dram_tensor("wt0", (128, QG * O), F16, kind="ExternalInput").ap()
    wt8 = nc.dram_tensor("wt8", (2, 128, NP8 * O), F8, kind="ExternalInput").ap()
    wt16 = nc.dram_tensor("wt16", (2, 128, NF16 * O), F16, kind="ExternalInput").ap()
    biases = nc.dram_tensor("biases", (128, 8), F32, kind="ExternalInput").ap()
    out = nc.dram_tensor("out", (128, 4, b_loc), F32, kind="ExternalOutput").ap()

    with tile.TileContext(nc) as tc, ExitStack() as ctx:
        const = ctx.enter_context(tc.tile_pool(name="const", bufs=1))
        t_pool = ctx.enter_context(tc.tile_pool(name="tpool", bufs=3))
        r0_pool = ctx.enter_context(tc.tile_pool(name="r0pool", bufs=2))
        rhs_pool = ctx.enter_context(tc.tile_pool(name="rhspool", bufs=2))
        tmp_pool = ctx.enter_context(tc.tile_pool(name="tmppool", bufs=2))
        hid_pool = ctx.enter_context(tc.tile_pool(name="hidpool", bufs=2))
        dt_pool = ctx.enter_context(tc.tile_pool(name="dtpool", bufs=3))
        psum_pool = ctx.enter_context(tc.tile_pool(name="psum", bufs=8, space="PSUM"))

        # --- resident tensors ---
        wt0_sb = const.tile([128, QG, O], F16)
        wt8_sb = [const.tile([128, NPAIRS8, 2, O], F8, name=f"w8_{l}") for l in range(2)]
        wt16_sb = [const.tile([128, NF16, O], F16, name=f"w16_{l}") for l in range(2)]
        bias_sb = const.tile([128, 8], F32)
        out_sb = const.tile([128, 4, b_loc], F32)

        nc.sync.dma_start(bias_sb[:], biases)
        nc.sync.dma_start(wt0_sb[:], wt0.rearrange("p (g o) -> p g o", o=O))
        # PE warmup: dep-free matmuls keep the p-state high through the
        # input-load window.
        warm_ps = psum_pool.tile([128, nc_cols], F32, tag="ps", name="warm_ps")
        for _ in range(72):
            nc.tensor.matmul(
                warm_ps[0:8, 0:8], bias_sb[:, 0:8], bias_sb[:, 0:8],
                start=True, stop=True,
            )
        for l in range(2):
            nc.scalar.dma_start(
                wt8_sb[l][:],
                wt8[l].rearrange("p (g j o) -> p g j o", g=NPAIRS8, j=2),
            )
            nc.scalar.dma_start(
                wt16_sb[l][:], wt16[l].rearrange("p (f o) -> p f o", o=O)
            )

        def load_T(j):
            """x0t rows broadcast to 128 partitions, split across two rings."""
            t_t = t_pool.tile([128, F, nc_cols], F16, tag="T", name=f"t_{j}")
            h1 = F // 2
            nc.sync.dma_start(
                t_t[:, 0:h1, :], xtc[j, 0:h1, :].partition_broadcast(128)
            )
            nc.scalar.dma_start(
                t_t[:, h1:F, :], xtc[j, h1:F, :].partition_broadcast(128)
            )
            return t_t

        def load_rhs0(j):
            r_t = r0_pool.tile([128, QG, nc_cols], F16, tag="r0", name=f"r0_{j}")
            nc.sync.dma_start(
                r_t[:], rhs0[j].rearrange("p (g i) -> p g i", i=nc_cols)
            )
            return r_t

        def mm_l0(j, r0_t, m, ps):
            for g in range(QG):
                nc.tensor.matmul(
                    ps[:],
                    wt0_sb[:, g, m * 128 : (m + 1) * 128],
                    r0_t[:, g, :],
                    start=(g == 0),
                    stop=(g == QG - 1),
                )

        def mm_l12(l, m, ps, rhs8_t, rhs16_t):
            """Hybrid matmul stream for layer l+1, output half m."""
            for g in range(NPAIRS8):
                nc.tensor.matmul(
                    ps[:],
                    wt8_sb[l][:, g, :, m * 128 : (m + 1) * 128],  # [128, 2, 128]
                    rhs8_t[:, g],                                 # [128, 2, nc]
                    start=(g == 0),
                    stop=False,
                    perf_mode=DR,
                )
            for t in range(NF16):
                nc.tensor.matmul(
                    ps[:],
                    wt16_sb[l][:, t, m * 128 : (m + 1) * 128],
                    rhs16_t[:, t, :],
                    start=False,
                    stop=(t == NF16 - 1),
                )

        def products(j, l, hid, t_t):
            """Build rhs8 [128, NPAIRS8, 2, nc] fp8 + rhs16 [128, NF16, nc]."""
            rhs8_t = rhs_pool.tile(
                [128, NPAIRS8, 2, nc_cols], F8, tag=f"rhs8_{l}", name=f"r8_{j}_{l}"
            )
            rhs8_flat = rhs8_t[:].rearrange("p g j i -> p (g j) i")
            rhs16_t = rhs_pool.tile(
                [128, NF16, nc_cols], F16, tag=f"rhs16_{l}", name=f"r16_{j}_{l}"
            )
            # GPSIMD: fp16 products for the last NGPS fp16-matmul fields
            nc.gpsimd.tensor_mul(
                rhs16_t[:, NF16 - NGPS :, :],
                hid[:].unsqueeze(1).broadcast_to((128, NGPS, nc_cols)),
                t_t[:, F - NGPS :, :],
            )
            # DVE (2x): fp16 products for the remaining fp16-matmul fields
            nc.vector.tensor_mul(
                rhs16_t[:, : NF16 - NGPS, :],
                hid[:].unsqueeze(1).broadcast_to((128, NF16 - NGPS, nc_cols)),
                t_t[:, NP8 : F - NGPS, :],
            )
            # two-step: DVE fp16 products (2x) in two groups -> ACT converts
            g1 = NTS // 2
            tmp_t = tmp_pool.tile(
                [128, NTS, nc_cols], F16, tag="tmp", bufs=3, name=f"tmp_{j}_{l}"
            )
            for a, bnd in ((0, g1), (g1, NTS)):
                nc.vector.tensor_mul(
                    tmp_t[:, a:bnd, :],
                    hid[:].unsqueeze(1).broadcast_to((128, bnd - a, nc_cols)),
                    t_t[:, a:bnd, :],
                )
                nc.scalar.activation(
                    rhs8_flat[:, a:bnd, :], tmp_t[:, a:bnd, :], AF.Copy
                )
            return rhs8_t, rhs16_t

        def hidden_act(j, l, ps):
            h_t = hid_pool.tile([128, nc_cols], F16, tag="hid", name=f"h_{j}_{l}")
            nc.scalar.activation(
                h_t[:], ps[:], AF.Relu,
                bias=bias_sb[:, 2 * l + 1 : 2 * l + 2], scale=1.0 / WS,
            )
            return h_t

        def direct_relu(j, slot, ps, bias_col, dt_t):
            # dt = relu(ps + 64*b) = 64 * relu(ps/64 + b); host divides by 64
            nc.scalar.activation(
                dt_t[:, slot, :], ps[:], AF.Relu,
                bias=bias_sb[:, bias_col : bias_col + 1],
            )

        # ---- prologue ----
        t_tiles = {0: load_T(0)}
        r0_tiles = {0: load_rhs0(0)}
        dt_tiles = {}
        state = {}   # per-chunk: rhs8/rhs16 tiles for next layer

        for i in range(nchunk + 2):
            k0, k1, k2 = i, i - 1, i - 2
            # prefetch next chunk inputs
            if k0 + 1 < nchunk:
                t_tiles[k0 + 1] = load_T(k0 + 1)
                r0_tiles[k0 + 1] = load_rhs0(k0 + 1)

            if k0 < nchunk:
                dt_t = dt_pool.tile(
                    [128, 4, nc_cols], F16, tag="dt", name=f"dt_{k0}"
                )
                dt_tiles[k0] = dt_t
                r0_t = r0_tiles.pop(k0)
                ps_b = psum_pool.tile([128, nc_cols], F32, tag="ps", name=f"ps_{k0}_0b")
                mm_l0(k0, r0_t, 1, ps_b)
                h0 = hidden_act(k0, 0, ps_b)
                ps_a = psum_pool.tile([128, nc_cols], F32, tag="ps", name=f"ps_{k0}_0a")
                mm_l0(k0, r0_t, 0, ps_a)
                state[(k0, 1)] = products(k0, 0, h0, t_tiles[k0])
                direct_relu(k0, 0, ps_a, 0, dt_t)

            if 0 <= k1 < nchunk:
                rhs8_t, rhs16_t = state.pop((k1, 1))
                ps1 = psum_pool.tile([128, nc_cols], F32, tag="ps", name=f"ps_{k1}_1b")
                mm_l12(0, 1, ps1, rhs8_t, rhs16_t)
                h1 = hidden_act(k1, 1, ps1)
                ps0 = psum_pool.tile([128, nc_cols], F32, tag="ps", name=f"ps_{k1}_1a")
                mm_l12(0, 0, ps0, rhs8_t, rhs16_t)
                state[(k1, 2)] = products(k1, 1, h1, t_tiles[k1])
                direct_relu(k1, 1, ps0, 2, dt_tiles[k1])

            if 0 <= k2 < nchunk:
                rhs8_t, rhs16_t = state.pop((k2, 2))
                ps20 = psum_pool.tile([128, nc_cols], F32, tag="ps", name=f"ps_{k2}_2a")
                mm_l12(1, 0, ps20, rhs8_t, rhs16_t)
                ps21 = psum_pool.tile([128, nc_cols], F32, tag="ps", name=f"ps_{k2}_2b")
                mm_l12(1, 1, ps21, rhs8_t, rhs16_t)
                dt_t = dt_tiles.pop(k2)
                direct_relu(k2, 2, ps20, 4, dt_t)
                direct_relu(k2, 3, ps21, 5, dt_t)
                nc.vector.tensor_reduce(
                    out_sb[:, :, k2 * nb : (k2 + 1) * nb],
                    dt_t[:].rearrange("p s (b d) -> p s b d", d=D),
                    axis=mybir.AxisListType.X,
                    op=mybir.AluOpType.add,
                )
                t_tiles.pop(k2, None)

        nc.sync.dma_start(out, out_sb[:])

    nc.compile()
    return nc


def _e4m3(a):
    a = np.clip(np.asarray(a, np.float32), -240.0, 240.0)
    return a.astype(ml_dtypes.float8_e4m3fn)


def _pack_inputs(field_embeddings, w0, b0, w1, b1, w2, b2, b_loc=B_LOC, nc_cols=NC):
    x = np.asarray(field_embeddings, dtype=np.float32)
    w0 = np.asarray(w0, dtype=np.float32)
    ncores = x.shape[0] // b_loc
    n_loc = b_loc * D
    nchunk = n_loc // nc_cols

    # layer-0 folded pairs
    hq = np.array([h for f_ in range(F) for h in range(f_ + 1)])
    fq = np.array([f_ for f_ in range(F) for h in range(f_ + 1)])
    w0r = w0.reshape(O, F, F)
    wf = w0r[:, hq, fq] + np.where(hq == fq, 0.0, w0r[:, fq, hq])   # (O, NPAIR)
    wf_pad = np.zeros((O, Q), dtype=np.float32)
    wf_pad[:, :NPAIR] = wf * WS
    wt0h = np.ascontiguousarray(
        wf_pad.reshape(O, QG, 128).transpose(2, 1, 0).reshape(128, QG * O)
    ).astype(np.float16)

    # layers 1/2: fp8 pair-packed weights [l][p=h, g, j, m, t] and fp16 tail
    def pack_l(w):
        a = np.asarray(w, np.float32).reshape(O, H, F).transpose(1, 2, 0) * WS  # (h,f,o)
        w8 = _e4m3(a[:, :NP8, :])                       # (h, NP8, O) = (p, g, j, o)
        w8 = np.ascontiguousarray(w8.reshape(H, NP8 * O))
        w16_ = np.ascontiguousarray(a[:, NP8:, :].reshape(H, NF16 * O)).astype(np.float16)
        return w8, w16_

    w8_1, w16_1 = pack_l(w1)
    w8_2, w16_2 = pack_l(w2)
    wt8h = np.stack([w8_1, w8_2])
    wt16h = np.stack([w16_1, w16_2])

    biash = np.zeros((128, 8), dtype=np.float32)
    for li, bvec in enumerate([b0, b1, b2]):
        bvec = np.asarray(bvec, dtype=np.float32)
        biash[:, 2 * li] = bvec[0:128] * WS          # direct half (kept scaled)
        biash[:, 2 * li + 1] = bvec[128:256]         # hidden half (scale=1/64)
    biash[:, 5] = np.asarray(b2, np.float32)[128:256] * WS  # L2 m1 is direct too

    in_maps = []
    for c in range(ncores):
        xc = x[c * b_loc : (c + 1) * b_loc]
        x0t = xc.transpose(1, 0, 2).reshape(F, n_loc)
        xtc = x0t.reshape(F, nchunk, nc_cols).transpose(1, 0, 2)
        x0t16 = x0t.astype(np.float16)
        # host-side layer-0 interactions (fp16 products of fp16 x)
        r0 = np.zeros((Q, n_loc), dtype=np.float16)
        r0[:NPAIR] = (x0t16[hq].astype(np.float32) * x0t16[fq].astype(np.float32)).astype(np.float16)
        r0p = r0.reshape(QG, 128, nchunk, nc_cols).transpose(2, 1, 0, 3)
        r0p = np.ascontiguousarray(r0p.reshape(nchunk, 128, QG * nc_cols))
        in_maps.append(
            {
                "xtc": np.ascontiguousarray(xtc).astype(np.float16),
                "rhs0": r0p,
                "wt0": wt0h,
                "wt8": wt8h,
                "wt16": wt16h,
                "biases": biash,
            }
        )
    return in_maps


_MODULE = None


def kernel(field_embeddings, w0, b0, w1, b1, w2, b2):
    global _MODULE, _LAST_RESULTS
    if _MODULE is None:
        _MODULE = build_module()
    nc = _MODULE
    in_maps = _pack_inputs(field_embeddings, w0, b0, w1, b1, w2, b2)
    res = bass_utils.run_bass_kernel_spmd(
        nc, in_maps, core_ids=list(range(NCORES)), trace=TRACE
    )
    _LAST_RESULTS = res
    outs = []
    for c in range(NCORES):
        o = res.results[c]["out"]                  # (128, 4, B_LOC) fp32
        full = o.transpose(1, 0, 2).reshape(512, B_LOC) * (1.0 / WS)
        outs.append(full.T)                        # (B_LOC, 512)
    return np.ascontiguousarray(np.concatenate(outs, axis=0), dtype=np.float32)
